# revision 9
# baseline (speedup 1.0000x reference)
"""CPC / NT-Xent loss kernel for 8 Trainium2 NeuronCores.

Reference computation (x, y: [8192, 256] f32):
    x_norm, y_norm = L2-normalized rows
    xy = concat(x_norm, y_norm)            # [16384, 256]
    sim = xy @ xy.T                        # [16384, 16384]
    denom_i = sum_j exp(sim_ij / tau) - exp(sim_ii / tau)
    pos_i   = dot(xy_i, yx_i)  (yx = concat(y_norm, x_norm))
    loss = mean( log(denom_i) - pos_i / tau )

Sharding: the 2N=16384 rows are data-parallel across the 8 cores.  Each
core receives the full row matrix ROTATED so its own 2048 rows sit at
local rows 0..2048 — the kernel is then a single SPMD program with no
core-dependent control flow.  The partner row (for pos_i) of local row i
is always local row 8192+i, independent of the rotation.

Per core, fully fused on-chip (the 16384x16384 sim matrix never touches
HBM):
  - load b [16384, 256] f32, row tiles [128, 256]
  - ss = row sums of squares (DVE tensor_tensor_reduce)
  - inv = exp(-0.5 * ln(ss))  (ACT; keeps everything in the single
    natural_log_exp activation-table set - no table swaps)
  - rows scaled to unit norm (f32), PE-transposed (f32) into PSUM,
    copied to SBUF as bf16 B_T [256, 16384]
  - Gram row-block: for each m-tile (128 rows) x 2048-col chunk:
    bf16 matmul (K=256 = 2 accum steps) -> PSUM f32
  - ACT exp(2*psum) with accum_out giving the row-chunk sums directly
  - denominator = rowsum - e^2  (sim_ii == 1 exactly)
  - nt_xent = ln(denominator) - 2*pos ; written out per row
Host: concatenates the 8 x 2048 per-row losses and takes the mean.
"""

import numpy as np
from contextlib import ExitStack

import concourse.bacc as bacc
import concourse.bass as bass
import concourse.tile as tile
import concourse.mybir as mybir
from concourse import bass_utils
from concourse.masks import make_identity

F32 = mybir.dt.float32
BF16 = mybir.dt.bfloat16
AF = mybir.ActivationFunctionType
ALU = mybir.AluOpType

P = 128          # partitions
TAU = 0.5
N_CORES = 8

# Full-problem geometry (hardcoded per contract)
B_ROWS = 8192    # rows in x (and y)
H = 256          # feature dim (= 2 k-tiles of 128)
N_TOTAL = 2 * B_ROWS          # 16384 rows of the concat matrix
N_MINE = N_TOTAL // N_CORES   # 2048 rows per core
CHUNK = 2048                  # columns processed per outer step (4 PSUM banks f32)


def build_program(n_total=N_TOTAL, n_mine=N_MINE, chunk=CHUNK, repeat=1,
                  enable_asserts=False):
    """Build the SPMD Bass program. Returns (nc, in_name, out_name).

    repeat>1 re-runs the whole computation sequentially (same result) —
    used to measure device time differentially when NTFF tracing is
    unavailable.
    """
    T = n_total // P              # total row tiles
    MT = n_mine // P              # my row tiles (M dimension)
    TPG = chunk // P              # row tiles ingested per outer step
    G = n_total // chunk          # outer steps
    NJ = chunk // 512             # 512-wide matmul slices per chunk
    half = T // 2                 # partner offset, in tiles
    assert H == 2 * P and half >= MT and chunk % 512 == 0
    assert n_total % chunk == 0 and n_mine % P == 0
    assert MT * P <= chunk        # lhsT slices live in the group-0 BT tile

    nc = bacc.Bacc(
        "TRN2",
        target_bir_lowering=False,
        debug=False,
        enable_asserts=enable_asserts,
        num_devices=N_CORES,
    )
    b_dram = nc.dram_tensor("b", [n_total, H], F32, kind="ExternalInput")
    nt_dram = nc.dram_tensor("nt", [P, MT], F32, kind="ExternalOutput")
    b_ap = b_dram.ap()

    with ExitStack() as ctx:
        tc = ctx.enter_context(tile.TileContext(nc))

        const_pool = ctx.enter_context(tc.tile_pool(name="const", bufs=1))
        bt_pool = ctx.enter_context(tc.tile_pool(name="bt", bufs=1))
        stat_pool = ctx.enter_context(tc.tile_pool(name="stat", bufs=1))
        load_pool = ctx.enter_context(tc.tile_pool(name="load", bufs=TPG + 2))
        keep_pool = ctx.enter_context(tc.tile_pool(name="keep", bufs=MT))
        nrm_pool = ctx.enter_context(tc.tile_pool(name="nrm", bufs=4))
        sq_pool = ctx.enter_context(tc.tile_pool(name="sq", bufs=2))
        ln_pool = ctx.enter_context(tc.tile_pool(name="ln", bufs=2))
        exp_pool = ctx.enter_context(tc.tile_pool(name="expo", bufs=3))
        psum_pool = ctx.enter_context(tc.tile_pool(name="ps", bufs=2, space="PSUM"))
        fin_pool = ctx.enter_context(tc.tile_pool(name="fin", bufs=1))

        identity = const_pool.tile([P, P], F32)
        make_identity(nc, identity[:])

        # B_T: normalized rows, transposed, bf16.  Two k-halves, one tile
        # per column group (separate tiles -> no false cross-group deps).
        BT0s = [bt_pool.tile([P, chunk], BF16, tag=f"bt0_{g}", name=f"bt0_{g}")
                for g in range(G)]
        BT1s = [bt_pool.tile([P, chunk], BF16, tag=f"bt1_{g}", name=f"bt1_{g}")
                for g in range(G)]

        ss_all = stat_pool.tile([P, T], F32)     # col ti = sum(b_tile_ti^2) per row
        inv_all = stat_pool.tile([P, T], F32)    # 1/norm
        rs_all = stat_pool.tile([P, MT * G], F32)  # exp row-chunk sums, col = m*G+g
        pos_all = stat_pool.tile([P, MT], F32)   # pos_sim per my-row

        for rep in range(repeat):
            run_body(nc, tc, b_ap, nt_dram, identity, BT0s, BT1s,
                     ss_all, inv_all, rs_all, pos_all,
                     load_pool, keep_pool, nrm_pool, sq_pool, ln_pool,
                     exp_pool, psum_pool, fin_pool,
                     T, MT, TPG, G, NJ, half, chunk)

    nc.compile()
    return nc, "b", "nt"


def run_body(nc, tc, b_ap, nt_dram, identity, BT0s, BT1s,
             ss_all, inv_all, rs_all, pos_all,
             load_pool, keep_pool, nrm_pool, sq_pool, ln_pool,
             exp_pool, psum_pool, fin_pool,
             T, MT, TPG, G, NJ, half, chunk):
        kept = [None] * MT  # f32 normalized tiles of my rows

        for g in range(G):
            # ---- ingest row tiles [g*TPG, (g+1)*TPG) ----
            raws = []
            for t in range(TPG):
                ti = g * TPG + t
                raw = load_pool.tile([P, H], F32)
                nc.sync.dma_start(out=raw[:], in_=b_ap[ti * P:(ti + 1) * P, :])
                sq = sq_pool.tile([P, H], F32)
                nc.vector.scalar_tensor_tensor(
                    out=sq[:], in0=raw[:], scalar=1.0, in1=raw[:],
                    op0=ALU.mult, op1=ALU.mult,
                    accum_out=ss_all[:, ti:ti + 1],
                )
                raws.append(raw)
            # inv_norm = exp(-0.5 * ln(ss)) — single act-table set
            lns = ln_pool.tile([P, TPG], F32)
            nc.scalar.activation(
                out=lns[:], in_=ss_all[:, g * TPG:(g + 1) * TPG], func=AF.Ln)
            nc.scalar.activation(
                out=inv_all[:, g * TPG:(g + 1) * TPG], in_=lns[:],
                func=AF.Exp, scale=-0.5)

            # ---- normalize (f32) + transpose into PSUM collectors ----
            psA = psum_pool.tile([P, chunk], F32, tag="ps")
            psB = psum_pool.tile([P, chunk], F32, tag="ps")
            for t in range(TPG):
                ti = g * TPG + t
                pool = keep_pool if ti < MT else nrm_pool
                nrm = pool.tile([P, H], F32)
                nc.vector.tensor_scalar_mul(nrm[:], raws[t][:], inv_all[:, ti:ti + 1])
                if ti < MT:
                    kept[ti] = nrm
                if half <= ti < half + MT:
                    m = ti - half
                    sq2 = sq_pool.tile([P, H], F32)
                    nc.vector.scalar_tensor_tensor(
                        out=sq2[:], in0=nrm[:], scalar=1.0, in1=kept[m][:],
                        op0=ALU.mult, op1=ALU.mult,
                        accum_out=pos_all[:, m:m + 1],
                    )
                nc.tensor.transpose(
                    psA[:, t * P:(t + 1) * P], nrm[:, 0:P], identity[:])
                nc.tensor.transpose(
                    psB[:, t * P:(t + 1) * P], nrm[:, P:2 * P], identity[:])
            # copy PSUM collectors into B_T (cast f32 -> bf16)
            nc.vector.tensor_copy(out=BT0s[g][:], in_=psA[:])
            nc.vector.tensor_copy(out=BT1s[g][:], in_=psB[:])

            # ---- Gram row-block x this column chunk ----
            for m in range(MT):
                ps = psum_pool.tile([P, chunk], F32, tag="ps")
                lhs0 = BT0s[0][:, m * P:(m + 1) * P]
                lhs1 = BT1s[0][:, m * P:(m + 1) * P]
                for j in range(NJ):
                    nc.tensor.matmul(
                        ps[:, j * 512:(j + 1) * 512], lhs0,
                        BT0s[g][:, j * 512:(j + 1) * 512],
                        start=True, stop=False)
                for j in range(NJ):
                    nc.tensor.matmul(
                        ps[:, j * 512:(j + 1) * 512], lhs1,
                        BT1s[g][:, j * 512:(j + 1) * 512],
                        start=False, stop=True)
                eo = exp_pool.tile([P, chunk], BF16)
                nc.scalar.activation(
                    out=eo[:], in_=ps[:], func=AF.Exp, scale=2.0,
                    accum_out=rs_all[:, m * G + g: m * G + g + 1])

        # ---- finalize ----
        rowsum = fin_pool.tile([P, MT], F32)
        nc.vector.tensor_reduce(
            out=rowsum[:], in_=rs_all[:].rearrange("p (m g) -> p m g", g=G),
            axis=mybir.AxisListType.X, op=ALU.add)
        denom = fin_pool.tile([P, MT], F32)
        nc.vector.tensor_scalar_add(denom[:], rowsum[:], -float(np.exp(2.0)))
        lnd = fin_pool.tile([P, MT], F32)
        nc.scalar.activation(out=lnd[:], in_=denom[:], func=AF.Ln)
        ntv = fin_pool.tile([P, MT], F32)
        # nt = (pos * -2) + ln(denom)
        nc.vector.scalar_tensor_tensor(
            out=ntv[:], in0=pos_all[:], scalar=-2.0, in1=lnd[:],
            op0=ALU.mult, op1=ALU.add)
        nc.sync.dma_start(out=nt_dram.ap(), in_=ntv[:])


_CACHE = {}


def _get_program():
    if "nc" not in _CACHE:
        _CACHE["nc"] = build_program()
    return _CACHE["nc"]


def kernel(x: np.ndarray, y: np.ndarray) -> np.ndarray:
    x = np.asarray(x, dtype=np.float32)
    y = np.asarray(y, dtype=np.float32)
    xy = np.concatenate([x, y], axis=0)          # [16384, 256]

    nc, in_name, out_name = _get_program()

    in_maps = []
    for c in range(N_CORES):
        off = c * N_MINE
        b_rot = np.ascontiguousarray(np.roll(xy, -off, axis=0))
        in_maps.append({in_name: b_rot})

    res = bass_utils.run_bass_kernel_spmd(
        nc, in_maps, core_ids=list(range(N_CORES)))

    # nt[c][p, m] = loss for global row (c*N_MINE + m*128 + p)
    rows = np.concatenate(
        [res.results[c][out_name].T.reshape(-1) for c in range(N_CORES)])
    loss = rows.astype(np.float64).mean()
    return np.float32(loss)
